# revision 29
# baseline (speedup 1.0000x reference)
"""Trainium2 kernel for the NNUE-style factorized embedding segment-sum.

Strategy ("flip"): the ragged two-table embedding-bag is reformulated as
block matmuls with the TABLE STATIONARY and the per-bag count columns as the
moving operand.  For output row (bag) with king-square block t, the
gather+segment-sum over its ragged feature ids equals
merged_table[t] slices.T @ count_cols, computed as 6 contraction chunks
(768 = 6*128 features) x 2 dout halves accumulating in
PSUM[dout_half=128, items].

Wins over the chunked lhsT=counts formulation (42.7us baseline):
 - items stream as matmul COLUMNS, so per-(core,slot) capacity is the max
   block size rounded to 8, not to 128: ~2% padding instead of ~12.5%, and
   the PE does 12 cycles/item (the dense floor: 768*256/128^2) -> ~21us.
 - counts ride as fp8e4 (ints <= 15 exact; the Poisson(0.04) counts max at
   3, anything above is clipped and those rows recomputed on host), with
   one stream pair (j2, j5) nibble-packed to shave HBM bytes: whole-slot
   uint32 bitwise split on DVE (only DVE passes the bitVec engine check)
   + one u8->fp8 cast on ACT.  Packing more pairs does not fit: Pool
   casts at 2.5ns/elem, ACT at 1.2, and DVE is full with PSUM drains.
 - outputs ride to HBM as uint8 (tables pre-scaled by 255, PSUM clipped to
   [0,255] on the DVE drain, host divides back): total chip HBM traffic
   ~46MB, well under the chip ceiling for the ~22us window.
 - dummy matmuls from t~0 keep the PE busy through the DMA fill so the HAM
   clock gate (1.2 -> 2.4 GHz after one fully-busy ~3.4us window) flips
   early in the stream instead of ~8us in.

Host (integer work only): merge factor tables (tiles+(pieces+ranks+files)
*mask -> [64,768,256] fp16), build per-bag count rows in merged-table space
(output b columns flip-remapped so ONE table serves both outputs), sort the
64 blocks by descending item count and deal rank r -> (slot r//8, core r%8)
so each slot's shared SPMD capacity is tight, pack counts, unscramble
outputs.

Device per slot: load table tile (ACT ring) + packed counts (SP ring,
issued two slots ahead, paired for line rate), 2 unpack ops per item group,
24 matmuls (2 item groups x 2 dout halves x 6 feature chunks), clipped
PSUM->u8 drains on DVE, stores batched per slot-pair mid-kernel with the
final slot's pieces split tiny across both rings (the exit barrier gates on
their HBM write receipts).
"""

import numpy as np
import ml_dtypes

import concourse.bass as bass
import concourse.tile as tile
from concourse import bacc, mybir
from concourse.bass_utils import run_bass_kernel_spmd

N_CORES = 8
B = 16384          # bags
KPL = 12           # piece planes
DOUT = 256
PIECE = 768        # KPL * 64
NBLK = 8           # table blocks (slots) per core: 64 king squares / 8 cores
TABW = 12 * 128    # table tile cols per slot: (2 dout halves x 6 fchunks) x 128
OSCALE = 255.0     # uint8 output quantization scale (folded into the table)

# ---------------------------------------------------------------------------
# host-side integer prep tables
_sq = np.arange(64)
_PERM = (7 - _sq // 8) * 8 + _sq % 8          # vertical king-square flip
_v = np.arange(PIECE)
_vk, _vr, _vf = _v // 64, (_v % 64) // 8, _v % 8
_FLIP_COL = ((_vk + 6) % 12) * 64 + (7 - _vr) * 8 + _vf

_prog_cache = {}

NDUMMY = 38        # PE warmup matmuls riding the DMA fill window


def _groups(cap):
    g0 = (cap // 2 + 7) // 8 * 8
    return [(0, g0), (g0, cap - g0)]


def _build_program(caps: tuple):
    """Bass program for one core (SPMD across all 8).

    caps[s] = item capacity of slot s (multiple of 8, shared by all cores).
    """
    nc = bacc.Bacc("TRN2", target_bir_lowering=False, debug=False)
    f32 = mybir.dt.float32
    fp16 = mybir.dt.float16
    fp8 = mybir.dt.float8e4
    u8 = mybir.dt.uint8
    u32 = mybir.dt.uint32
    A = mybir.AluOpType

    sum_caps = sum(caps)
    cm_w = [5 * c for c in caps]       # count bytes per slot (4 plain + 1 packed)
    base = np.concatenate([[0], np.cumsum(caps)]).astype(int)
    cbase = np.concatenate([[0], np.cumsum(cm_w)]).astype(int)
    maxcap = max(caps)

    tab = nc.dram_tensor("tab", [128, NBLK * TABW], fp16,
                         kind="ExternalInput").ap()
    # cm[p, cbase(s) + goff*3 + jj*gn + m] = count[item m, feat jj*128+p]
    #   + 16*count[item m, feat (jj+3)*128+p]   (group-major within a slot)
    cm = nc.dram_tensor("cm", [128, int(cbase[-1])], u8,
                        kind="ExternalInput").ap()
    # out[p, 2*base(s) + h*cap + m] = round(255*clip(result))[item m, h*128+p]
    out = nc.dram_tensor("out", [128, 2 * sum_caps], u8,
                         kind="ExternalOutput").ap()

    with tile.TileContext(nc) as tc:
        with (
            tc.tile_pool(name="tabp", bufs=5) as tabp,
            tc.tile_pool(name="cmp", bufs=4) as cmp_,
            tc.tile_pool(name="cmup", bufs=3) as cmup,
            tc.tile_pool(name="cmtp", bufs=3) as cmtp,
            tc.tile_pool(name="outp", bufs=4) as outp,
            tc.tile_pool(name="warmp", bufs=1) as wmp,
            tc.tile_pool(name="ps", bufs=8, space="PSUM") as psp,
        ):
            # ---- PE warmup: HAM clock gate needs ~3.4us of sustained busy
            # time to lift 1.2 -> 2.4 GHz; burn the DMA fill window on dummy
            # matmuls so the real stream runs warm.  Memsets ride DVE (idle
            # at start - GpSimd's queue carries framework memsets).
            wl = wmp.tile([128, 128], fp16, tag="warml")
            wr = wmp.tile([128, 128], fp16, tag="warmr")
            nc.vector.memset(wl[:], 0)
            nc.vector.memset(wr[:], 0)
            wp = psp.tile([128, 512], f32, tag="ps")
            for _ in range(NDUMMY):
                nc.tensor.matmul(wp[:, :128], lhsT=wl[:], rhs=wr[:])

            # ---- DMA plan: tables on the ACT HWDGE ring, packed counts on
            # the SP ring.  Slot 0 splits at the dout-half / item-group
            # boundary so the first matmuls wait on less data; later slots
            # load in pairs (bigger DMAs run closer to line rate, half the
            # ring issue slots).
            tts = {}

            def load_tab(s0, nslots=1, split=1):
                tt = tabp.tile([128, nslots * TABW], fp16, tag="tab")
                w = nslots * TABW
                bnds = [w * k // split // 128 * 128 for k in range(split + 1)]
                for k in range(split):
                    nc.scalar.dma_start(
                        tt[:, bnds[k]:bnds[k + 1]],
                        tab[:, s0 * TABW + bnds[k]:s0 * TABW + bnds[k + 1]])
                for q in range(nslots):
                    tts[s0 + q] = (tt, q * TABW)

            cms = {}

            def load_cm(s0, nslots=1, bnds=None, order=None):
                w = int(cbase[s0 + nslots] - cbase[s0])
                ct = cmp_.tile([128, w], u8, tag="cm")
                if bnds is None:
                    bnds = [0, w]
                pieces = list(range(len(bnds) - 1))
                if order is not None:
                    pieces = order
                for k in pieces:
                    nc.sync.dma_start(
                        ct[:, bnds[k]:bnds[k + 1]],
                        cm[:, int(cbase[s0]) + bnds[k]:
                           int(cbase[s0]) + bnds[k + 1]])
                off = 0
                for q in range(nslots):
                    cms[s0 + q] = (ct, off)
                    off += cm_w[s0 + q]

            g0n = _groups(caps[0])[0][1]
            c0 = caps[0]
            load_tab(0, split=2)
            load_cm(0, bnds=[0, 4 * g0n, 4 * c0, 5 * c0],
                    order=[0, 2, 1])
            g1n = _groups(caps[1])[0][1]
            c1 = caps[1]
            load_tab(1, split=2)
            load_cm(1, bnds=[0, 4 * g1n, 4 * c1, 5 * c1],
                    order=[0, 2, 1])

            outt = None
            for s in range(NBLK):
                cap = caps[s]
                tt, toff = tts[s]
                ct, coff = cms[s]

                # ---- unpack the (j2, j5) nibble pair.  The other four
                # streams ride as plain fp8 straight from HBM (engines lack
                # the elementwise budget to cast more than ~one stream per
                # slot: Pool casts at 2.5ns/elem, ACT at 1.2, DVE is full
                # with drains).  Whole-slot uint32 bitwise split on DVE
                # (only DVE passes the bitVec engine check), one u8->fp8
                # cast on ACT over both nibble streams.
                cmu = cmup.tile([128, 2 * maxcap], u8, tag="cmu")
                cmt = cmtp.tile([128, 2 * maxcap], fp8, tag="cmt")
                x32 = ct[:, coff + 4 * cap:coff + 5 * cap].bitcast(u32)
                nc.vector.tensor_scalar(cmu[:, 0:cap].bitcast(u32), x32,
                                        0x0F0F0F0F, None, A.bitwise_and)
                nc.vector.tensor_scalar(cmu[:, cap:2 * cap].bitcast(u32),
                                        x32, 4, 0x0F0F0F0F,
                                        A.logical_shift_right,
                                        A.bitwise_and)
                # j2 cast on DVE (needed by the 5th matmul of each
                # group; ACT's queue carries the DMA issues and would
                # deliver it late), j5 cast on ACT (needed by the 6th)
                nc.vector.tensor_scalar(cmt[:, :cap], cmu[:, :cap],
                                        1.0, None, A.mult)
                nc.scalar.activation(cmt[:, cap:2 * cap],
                                     cmu[:, cap:2 * cap],
                                     mybir.ActivationFunctionType.Copy)

                # prefetch (paired), two slots ahead
                ns = max(tts.keys()) + 1
                while ns < NBLK and ns <= s + 3:
                    k = min(2, NBLK - ns)
                    load_tab(ns, nslots=k)
                    load_cm(ns, nslots=k)
                    ns += k

                # ---- matmuls: table tile stationary, count cols moving;
                # slot-pair-shared out tile (even slot first half)
                if s % 2 == 0:
                    pair_w = 2 * (cap + (caps[s + 1] if s + 1 < NBLK else 0))
                    outt = outp.tile([128, 2 * 2 * maxcap], u8, tag="out")
                    ooff = 0
                else:
                    ooff = 2 * caps[s - 1]
                g0n_ = _groups(cap)[0][1]
                pbase = [0, 4 * g0n_]      # plain-region group offsets
                qbase = [0, g0n_]          # nibble-region group offsets
                PLAIN_IDX = {0: 0, 1: 1, 3: 2, 4: 3}
                # slot 0 runs group-outer so the last-loaded plain-g1
                # piece is not needed until halfway through the slot
                hg = ([(h, gg) for h in (0, 1)
                       for gg in enumerate(_groups(cap))] if s > 0 else
                      [(h, gg) for gg in enumerate(_groups(cap))
                       for h in (0, 1)])
                for h, (gi, (goff, gn)) in hg:
                        ps = psp.tile([128, 512], f32, tag="ps")
                        # plain streams first: the packed pair's unpack
                        # chain (load -> DVE bitwise -> cast) gets four
                        # matmuls of cover
                        for idx, j in enumerate((0, 1, 3, 4, 2, 5)):
                            if j in PLAIN_IDX:
                                cb = coff + pbase[gi] + PLAIN_IDX[j] * gn
                                rhs = ct[:, cb:cb + gn].bitcast(fp8)
                            else:
                                cb = (qbase[gi] if j == 2
                                      else cap + qbase[gi])
                                rhs = cmt[:, cb:cb + gn]
                            nc.tensor.matmul(
                                ps[:, :gn],
                                lhsT=tt[:, toff + (h * 6 + j) * 128:
                                        toff + (h * 6 + j + 1) * 128],
                                rhs=rhs,
                                start=(idx == 0),
                                stop=(idx == 5),
                            )
                        # clip(psum, 0, 255) -> u8 out tile (table carries
                        # the 255x scale; host divides back)
                        dsl = outt[:, ooff + h * cap + goff:
                                   ooff + h * cap + goff + gn]
                        nc.vector.tensor_scalar(dsl, ps[:, :gn],
                                                255.0, 0.0, A.min, A.max)
                        if s == 7 and h == 0 and gi == 1:
                            # final slot: h0 half fully drained - store it
                            # now so only the small h1 pieces gate the exit
                            ob7 = 2 * int(base[s])
                            nc.scalar.dma_start(
                                out[:, ob7:ob7 + cap],
                                outt[:, ooff:ooff + cap])

                # ---- stores: batched per slot pair for slots 0-5 (rings
                # alternate), singles for 6 and 7 with the final pieces
                # split small across both rings so their HBM write receipts
                # (which gate the exit barrier) overlap and cover little
                # data
                if s in (1, 3, 5):
                    w = 2 * (caps[s - 1] + cap)
                    ob = 2 * int(base[s - 1])
                    ring = nc.scalar if s in (1, 5) else nc.sync
                    ring.dma_start(out[:, ob:ob + w], outt[:, :w])
                elif s == 6:
                    ob = 2 * int(base[s])
                    nc.sync.dma_start(out[:, ob:ob + 2 * cap],
                                      outt[:, :2 * cap])
                elif s == 7:
                    ob = 2 * int(base[s])
                    cut = 2 * cap - 64
                    nc.sync.dma_start(out[:, ob + cap:ob + cut],
                                      outt[:, ooff + cap:ooff + cut])
                    nc.scalar.dma_start(out[:, ob + cut:ob + 2 * cap],
                                        outt[:, ooff + cut:ooff + 2 * cap])

    nc.compile()
    return nc


def _prep(values, lengths, kings):
    """Host prep: counts in merged-table column space, block assignment."""
    values = np.asarray(values).astype(np.int64)
    lengths = np.asarray(lengths).astype(np.int64)
    kings = np.asarray(kings).astype(np.int64)
    nb = lengths.shape[0]

    seg = np.repeat(np.arange(nb, dtype=np.int64), lengths)
    cnt_a = np.bincount(seg * PIECE + values,
                        minlength=nb * PIECE).reshape(nb, PIECE)
    cnt_b = np.bincount(seg * PIECE + _FLIP_COL[values],
                        minlength=nb * PIECE).reshape(nb, PIECE)
    cnt = np.concatenate([cnt_a, cnt_b])        # [2B, PIECE] ints

    blk = np.concatenate([kings[:, 0], _PERM[kings[:, 1]]])
    sizes = np.bincount(blk, minlength=64)
    order = np.argsort(blk, kind="stable")
    offs = np.concatenate([[0], np.cumsum(sizes)])

    rank = np.argsort(-sizes, kind="stable")    # block ids, desc size
    caps = tuple(int((sizes[rank[s * N_CORES]] + 7) // 8 * 8) or 8
                 for s in range(NBLK))
    base = np.concatenate([[0], np.cumsum(caps)]).astype(int)
    sum_caps = int(base[-1])

    pad_idx = np.full((N_CORES, sum_caps), -1, np.int64)
    for s in range(NBLK):
        for c in range(N_CORES):
            t = rank[s * N_CORES + c]
            ids = order[offs[t]:offs[t + 1]]
            pad_idx[c, base[s]:base[s] + len(ids)] = ids

    blk_of = rank.reshape(NBLK, N_CORES).T      # [core, slot]

    # rows where any clipped count would corrupt the result -> host recompute
    over_rows = np.unique(np.argwhere(cnt > 15)[:, 0])
    return cnt, pad_idx, caps, blk_of, over_rows


def _merged_tables(pieces, ranks, files, tiles, mask):
    p = np.asarray(pieces, np.float32)
    r = np.asarray(ranks, np.float32)
    f = np.asarray(files, np.float32)
    t = np.asarray(tiles, np.float32)
    m = np.asarray(mask, np.float32)
    return (t + (p + r + f) * m).reshape(64, PIECE, DOUT)


def _make_tab(merged, blk_of):
    """Per-core [128, NBLK*TABW] fp16, tile (s, h*6+j)[p, d] =
    255 * merged[blk(c,s), j*128+p, h*128+d]."""
    planes = (merged * OSCALE).reshape(64, 6, 128, 2, 128)
    planes = planes.astype(np.float16)
    tabs = []
    for c in range(N_CORES):
        tc_ = planes[blk_of[c]]                    # [8, 6, 128, 2, 128]
        tabs.append(np.ascontiguousarray(
            tc_.transpose(2, 0, 3, 1, 4).reshape(128, -1)))
    return tabs


def _make_cm(cnt_ext, pad_idx, caps):
    """Per-core count planes [128, sum(5*cap)] uint8.  Per slot:
    [plain g0: j0,j1,j3,j4 as fp8 bytes | plain g1 | packed g0: j2+16*j5 |
    packed g1]."""
    base = np.concatenate([[0], np.cumsum(caps)]).astype(int)
    fp8 = ml_dtypes.float8_e4m3
    cms = []
    for c in range(N_CORES):
        parts = []
        for s in range(NBLK):
            cap = caps[s]
            plain, packed = [], []
            for (goff, gn) in _groups(cap):
                ids = pad_idx[c, base[s] + goff:base[s] + goff + gn]
                sel = cnt_ext[ids]                     # [gn, 768] u8
                # [gn, 6, 128] -> [128(p), 6(j), gn(m)]
                t = sel.reshape(gn, 6, 128).transpose(2, 1, 0)
                plain.append(t[:, [0, 1, 3, 4]].astype(fp8).view(np.uint8)
                             .reshape(128, 4 * gn))
                packed.append(t[:, 2] + 16 * t[:, 5])  # [128, gn]
            parts += plain + packed
        cms.append(np.ascontiguousarray(np.concatenate(parts, axis=1)))
    return cms


def _run(inputs, trace=False):
    cnt, pad_idx, caps, blk_of, over_rows = _prep(
        inputs["values"], inputs["lengths"], inputs["kings"])

    merged = _merged_tables(inputs["pieces"], inputs["ranks"],
                            inputs["files"], inputs["tiles"],
                            inputs["factorization_mask"])

    key = caps
    if key not in _prog_cache:
        _prog_cache[key] = _build_program(caps)
    nc = _prog_cache[key]

    nb2 = cnt.shape[0]
    cnt_unclipped = cnt[over_rows].copy() if len(over_rows) else None
    np.minimum(cnt, 15, out=cnt, casting="unsafe")
    cnt_ext = np.zeros((nb2 + 1, PIECE), np.uint8)
    cnt_ext[:nb2] = cnt

    tabs = _make_tab(merged, blk_of)
    cms = _make_cm(cnt_ext, pad_idx, caps)
    in_maps = [{"tab": tabs[c], "cm": cms[c]} for c in range(N_CORES)]

    res = run_bass_kernel_spmd(nc, in_maps, list(range(N_CORES)),
                               trace=trace)

    base = np.concatenate([[0], np.cumsum(caps)]).astype(int)
    comb = np.zeros((nb2, DOUT), np.float32)
    for c in range(N_CORES):
        flat = res.results[c]["out"]               # [128, 2*sum_caps] u8
        for s in range(NBLK):
            cap = caps[s]
            # [128, 2, cap] -> [cap, 2, 128] -> [cap, 256]
            rows = (flat[:, 2 * base[s]:2 * (base[s] + cap)]
                    .astype(np.float32)
                    .reshape(128, 2, cap).transpose(2, 1, 0)
                    .reshape(cap, DOUT))
            ids = pad_idx[c, base[s]:base[s] + cap]
            valid = ids >= 0
            comb[ids[valid]] = rows[valid]
    comb *= np.float32(1.0 / OSCALE)
    np.clip(comb, 0.0, 1.0, out=comb)

    if len(over_rows):
        # counts were clipped at 15 on device for these rows: recompute f32
        kings = np.asarray(inputs["kings"]).astype(np.int64)
        row_blk = np.concatenate([kings[:, 0], _PERM[kings[:, 1]]])
        for i, r in enumerate(over_rows):
            comb[r] = np.clip(
                cnt_unclipped[i].astype(np.float32) @ merged[row_blk[r]],
                0.0, 1.0)

    return (comb[:B], comb[B:]), res


def kernel(**inputs):
    (a, b), _ = _run(inputs, trace=False)
    return a, b


# revision 31
# speedup vs baseline: 1.0099x; 1.0099x over previous
"""Trainium2 kernel for the NNUE-style factorized embedding segment-sum.

Strategy ("flip"): the ragged two-table embedding-bag is reformulated as
block matmuls with the TABLE STATIONARY and the per-bag count columns as the
moving operand.  For output row (bag) with king-square block t, the
gather+segment-sum over its ragged feature ids equals
merged_table[t] slices.T @ count_cols, computed as 6 contraction chunks
(768 = 6*128 features) x 2 dout halves accumulating in
PSUM[dout_half=128, items].

Wins over the chunked lhsT=counts formulation (42.7us baseline):
 - items stream as matmul COLUMNS, so per-(core,slot) capacity is the max
   block size rounded to 8, not to 128: ~2% padding instead of ~12.5%, and
   the PE does 12 cycles/item (the dense floor: 768*256/128^2) -> ~21us.
 - counts ride as fp8e4 (ints <= 15 exact; the Poisson(0.04) counts max at
   3, anything above is clipped and those rows recomputed on host), with
   one stream pair (j2, j5) nibble-packed to shave HBM bytes: whole-slot
   uint32 bitwise split on DVE (only DVE passes the bitVec engine check)
   + one u8->fp8 cast on ACT.  Packing more pairs does not fit: Pool
   casts at 2.5ns/elem, ACT at 1.2, and DVE is full with PSUM drains.
 - outputs ride to HBM as uint8 (tables pre-scaled by 255, PSUM clipped to
   [0,255] on the DVE drain, host divides back): total chip HBM traffic
   ~46MB, well under the chip ceiling for the ~22us window.
 - dummy matmuls from t~0 keep the PE busy through the DMA fill so the HAM
   clock gate (1.2 -> 2.4 GHz after one fully-busy ~3.4us window) flips
   early in the stream instead of ~8us in.

Host (integer work only): merge factor tables (tiles+(pieces+ranks+files)
*mask -> [64,768,256] fp16), build per-bag count rows in merged-table space
(output b columns flip-remapped so ONE table serves both outputs), sort the
64 blocks by descending item count and deal rank r -> (slot r//8, core r%8)
so each slot's shared SPMD capacity is tight, pack counts, unscramble
outputs.

Device per slot: load table tile (ACT ring) + packed counts (SP ring,
issued two slots ahead, paired for line rate), 2 unpack ops per item group,
24 matmuls (2 item groups x 2 dout halves x 6 feature chunks), clipped
PSUM->u8 drains on DVE, stores batched per slot-pair mid-kernel with the
final slot's pieces split tiny across both rings (the exit barrier gates on
their HBM write receipts).
"""

import numpy as np
import ml_dtypes

import concourse.bass as bass
import concourse.tile as tile
from concourse import bacc, mybir
from concourse.bass_utils import run_bass_kernel_spmd

N_CORES = 8
B = 16384          # bags
KPL = 12           # piece planes
DOUT = 256
PIECE = 768        # KPL * 64
NBLK = 8           # table blocks (slots) per core: 64 king squares / 8 cores
TABW = 12 * 128    # table tile cols per slot: (2 dout halves x 6 fchunks) x 128
OSCALE = 255.0     # uint8 output quantization scale (folded into the table)

# ---------------------------------------------------------------------------
# host-side integer prep tables
_sq = np.arange(64)
_PERM = (7 - _sq // 8) * 8 + _sq % 8          # vertical king-square flip
_v = np.arange(PIECE)
_vk, _vr, _vf = _v // 64, (_v % 64) // 8, _v % 8
_FLIP_COL = ((_vk + 6) % 12) * 64 + (7 - _vr) * 8 + _vf

_prog_cache = {}

NDUMMY = 38        # PE warmup matmuls riding the DMA fill window


def _groups(cap):
    g0 = (cap // 2 + 7) // 8 * 8
    return [(0, g0), (g0, cap - g0)]


def _build_program(caps: tuple):
    """Bass program for one core (SPMD across all 8).

    caps[s] = item capacity of slot s (multiple of 8, shared by all cores).
    """
    nc = bacc.Bacc("TRN2", target_bir_lowering=False, debug=False)
    f32 = mybir.dt.float32
    fp16 = mybir.dt.float16
    fp8 = mybir.dt.float8e4
    u8 = mybir.dt.uint8
    u32 = mybir.dt.uint32
    A = mybir.AluOpType

    sum_caps = sum(caps)
    cm_w = [5 * c for c in caps]       # count bytes per slot (4 plain + 1 packed)
    base = np.concatenate([[0], np.cumsum(caps)]).astype(int)
    cbase = np.concatenate([[0], np.cumsum(cm_w)]).astype(int)
    maxcap = max(caps)

    tab = nc.dram_tensor("tab", [128, NBLK * TABW], fp16,
                         kind="ExternalInput").ap()
    # cm[p, cbase(s) + goff*3 + jj*gn + m] = count[item m, feat jj*128+p]
    #   + 16*count[item m, feat (jj+3)*128+p]   (group-major within a slot)
    cm = nc.dram_tensor("cm", [128, int(cbase[-1])], u8,
                        kind="ExternalInput").ap()
    # out[p, 2*base(s) + h*cap + m] = round(255*clip(result))[item m, h*128+p]
    out = nc.dram_tensor("out", [128, 2 * sum_caps], u8,
                         kind="ExternalOutput").ap()

    with tile.TileContext(nc) as tc:
        with (
            tc.tile_pool(name="tabp", bufs=5) as tabp,
            tc.tile_pool(name="cmp", bufs=4) as cmp_,
            tc.tile_pool(name="cmup", bufs=3) as cmup,
            tc.tile_pool(name="cmtp", bufs=3) as cmtp,
            tc.tile_pool(name="outp", bufs=4) as outp,
            tc.tile_pool(name="warmp", bufs=1) as wmp,
            tc.tile_pool(name="ps", bufs=8, space="PSUM") as psp,
        ):
            # ---- PE warmup: HAM clock gate needs ~3.4us of sustained busy
            # time to lift 1.2 -> 2.4 GHz; burn the DMA fill window on dummy
            # matmuls so the real stream runs warm.  Memsets ride DVE (idle
            # at start - GpSimd's queue carries framework memsets).
            wl = wmp.tile([128, 128], fp16, tag="warml")
            wr = wmp.tile([128, 128], fp16, tag="warmr")
            nc.vector.memset(wl[:], 0)
            nc.vector.memset(wr[:], 0)
            wp = psp.tile([128, 512], f32, tag="ps")
            for _ in range(NDUMMY):
                nc.tensor.matmul(wp[:, :128], lhsT=wl[:], rhs=wr[:])

            # ---- DMA plan: tables on the ACT HWDGE ring, packed counts on
            # the SP ring.  Slot 0 splits at the dout-half / item-group
            # boundary so the first matmuls wait on less data; later slots
            # load in pairs (bigger DMAs run closer to line rate, half the
            # ring issue slots).
            tts = {}

            def load_tab(s0, nslots=1, split=1):
                tt = tabp.tile([128, nslots * TABW], fp16, tag="tab")
                w = nslots * TABW
                bnds = [w * k // split // 128 * 128 for k in range(split + 1)]
                for k in range(split):
                    nc.scalar.dma_start(
                        tt[:, bnds[k]:bnds[k + 1]],
                        tab[:, s0 * TABW + bnds[k]:s0 * TABW + bnds[k + 1]])
                for q in range(nslots):
                    tts[s0 + q] = (tt, q * TABW)

            cms = {}

            def load_cm(s0, nslots=1, bnds=None, order=None):
                w = int(cbase[s0 + nslots] - cbase[s0])
                ct = cmp_.tile([128, w], u8, tag="cm")
                if bnds is None:
                    bnds = [0, w]
                pieces = list(range(len(bnds) - 1))
                if order is not None:
                    pieces = order
                for k in pieces:
                    nc.sync.dma_start(
                        ct[:, bnds[k]:bnds[k + 1]],
                        cm[:, int(cbase[s0]) + bnds[k]:
                           int(cbase[s0]) + bnds[k + 1]])
                off = 0
                for q in range(nslots):
                    cms[s0 + q] = (ct, off)
                    off += cm_w[s0 + q]

            g0n = _groups(caps[0])[0][1]
            c0 = caps[0]
            load_tab(0, split=2)
            load_cm(0, bnds=[0, 4 * g0n, 4 * c0, 5 * c0],
                    order=[0, 2, 1])
            load_tab(1)
            load_cm(1)

            outt = None
            for s in range(NBLK):
                cap = caps[s]
                tt, toff = tts[s]
                ct, coff = cms[s]

                # ---- unpack the (j2, j5) nibble pair.  The other four
                # streams ride as plain fp8 straight from HBM (engines lack
                # the elementwise budget to cast more than ~one stream per
                # slot: Pool casts at 2.5ns/elem, ACT at 1.2, DVE is full
                # with drains).  Whole-slot uint32 bitwise split on DVE
                # (only DVE passes the bitVec engine check), one u8->fp8
                # cast on ACT over both nibble streams.
                cmu = cmup.tile([128, 2 * maxcap], u8, tag="cmu")
                cmt = cmtp.tile([128, 2 * maxcap], fp8, tag="cmt")
                x32 = ct[:, coff + 4 * cap:coff + 5 * cap].bitcast(u32)
                nc.vector.tensor_scalar(cmu[:, 0:cap].bitcast(u32), x32,
                                        0x0F0F0F0F, None, A.bitwise_and)
                nc.vector.tensor_scalar(cmu[:, cap:2 * cap].bitcast(u32),
                                        x32, 4, 0x0F0F0F0F,
                                        A.logical_shift_right,
                                        A.bitwise_and)
                # j2 cast on DVE (needed by the 5th matmul of each
                # group; ACT's queue carries the DMA issues and would
                # deliver it late), j5 cast on ACT (needed by the 6th)
                nc.vector.tensor_scalar(cmt[:, :cap], cmu[:, :cap],
                                        1.0, None, A.mult)
                nc.scalar.activation(cmt[:, cap:2 * cap],
                                     cmu[:, cap:2 * cap],
                                     mybir.ActivationFunctionType.Copy)

                # prefetch (paired), two slots ahead
                ns = max(tts.keys()) + 1
                while ns < NBLK and ns <= s + 3:
                    k = min(2, NBLK - ns)
                    load_tab(ns, nslots=k)
                    load_cm(ns, nslots=k)
                    ns += k

                # ---- matmuls: table tile stationary, count cols moving;
                # slot-pair-shared out tile (even slot first half)
                if s % 2 == 0:
                    pair_w = 2 * (cap + (caps[s + 1] if s + 1 < NBLK else 0))
                    outt = outp.tile([128, 2 * 2 * maxcap], u8, tag="out")
                    ooff = 0
                else:
                    ooff = 2 * caps[s - 1]
                g0n_ = _groups(cap)[0][1]
                pbase = [0, 4 * g0n_]      # plain-region group offsets
                qbase = [0, g0n_]          # nibble-region group offsets
                PLAIN_IDX = {0: 0, 1: 1, 3: 2, 4: 3}
                # slot 0 runs group-outer so the last-loaded plain-g1
                # piece is not needed until halfway through the slot
                hg = ([(h, gg) for h in (0, 1)
                       for gg in enumerate(_groups(cap))] if s > 0 else
                      [(h, gg) for gg in enumerate(_groups(cap))
                       for h in (0, 1)])
                for h, (gi, (goff, gn)) in hg:
                        ps = psp.tile([128, 512], f32, tag="ps")
                        # plain streams first: the packed pair's unpack
                        # chain (load -> DVE bitwise -> cast) gets four
                        # matmuls of cover
                        for idx, j in enumerate((0, 1, 3, 4, 2, 5)):
                            if j in PLAIN_IDX:
                                cb = coff + pbase[gi] + PLAIN_IDX[j] * gn
                                rhs = ct[:, cb:cb + gn].bitcast(fp8)
                            else:
                                cb = (qbase[gi] if j == 2
                                      else cap + qbase[gi])
                                rhs = cmt[:, cb:cb + gn]
                            nc.tensor.matmul(
                                ps[:, :gn],
                                lhsT=tt[:, toff + (h * 6 + j) * 128:
                                        toff + (h * 6 + j + 1) * 128],
                                rhs=rhs,
                                start=(idx == 0),
                                stop=(idx == 5),
                            )
                        # clip(psum, 0, 255) -> u8 out tile (table carries
                        # the 255x scale; host divides back)
                        dsl = outt[:, ooff + h * cap + goff:
                                   ooff + h * cap + goff + gn]
                        nc.vector.tensor_scalar(dsl, ps[:, :gn],
                                                255.0, 0.0, A.min, A.max)
                        if s == 7 and gi == 1 and h == 0:
                            # final slot: h0 half fully drained - store it
                            # now so only the small h1 pieces gate the exit
                            ob7 = 2 * int(base[s])
                            nc.scalar.dma_start(
                                out[:, ob7:ob7 + cap],
                                outt[:, ooff:ooff + cap])
                        if s == 7 and gi == 0 and h == 1:
                            # h1 group-0 region drained - store it early so
                            # the exit receipts cover only the g1 remainder
                            ob7 = 2 * int(base[s])
                            g0n7 = _groups(cap)[0][1]
                            nc.scalar.dma_start(
                                out[:, ob7 + cap:ob7 + cap + g0n7],
                                outt[:, ooff + cap:ooff + cap + g0n7])

                # ---- stores: batched per slot pair for slots 0-5 (rings
                # alternate), singles for 6 and 7 with the final pieces
                # split small across both rings so their HBM write receipts
                # (which gate the exit barrier) overlap and cover little
                # data
                if s in (1, 3, 5):
                    w = 2 * (caps[s - 1] + cap)
                    ob = 2 * int(base[s - 1])
                    ring = nc.scalar if s in (1, 5) else nc.sync
                    ring.dma_start(out[:, ob:ob + w], outt[:, :w])
                elif s == 6:
                    ob = 2 * int(base[s])
                    nc.sync.dma_start(out[:, ob:ob + 2 * cap],
                                      outt[:, :2 * cap])
                elif s == 7:
                    ob = 2 * int(base[s])
                    hb = cap + _groups(cap)[0][1]   # h1-g1 region start
                    cut = 2 * cap - 64
                    nc.sync.dma_start(out[:, ob + hb:ob + cut],
                                      outt[:, ooff + hb:ooff + cut])
                    nc.scalar.dma_start(out[:, ob + cut:ob + 2 * cap],
                                        outt[:, ooff + cut:ooff + 2 * cap])

    nc.compile()
    return nc


def _prep(values, lengths, kings):
    """Host prep: counts in merged-table column space, block assignment."""
    values = np.asarray(values).astype(np.int64)
    lengths = np.asarray(lengths).astype(np.int64)
    kings = np.asarray(kings).astype(np.int64)
    nb = lengths.shape[0]

    seg = np.repeat(np.arange(nb, dtype=np.int64), lengths)
    cnt_a = np.bincount(seg * PIECE + values,
                        minlength=nb * PIECE).reshape(nb, PIECE)
    cnt_b = np.bincount(seg * PIECE + _FLIP_COL[values],
                        minlength=nb * PIECE).reshape(nb, PIECE)
    cnt = np.concatenate([cnt_a, cnt_b])        # [2B, PIECE] ints

    blk = np.concatenate([kings[:, 0], _PERM[kings[:, 1]]])
    sizes = np.bincount(blk, minlength=64)
    order = np.argsort(blk, kind="stable")
    offs = np.concatenate([[0], np.cumsum(sizes)])

    rank = np.argsort(-sizes, kind="stable")    # block ids, desc size
    caps = tuple(int((sizes[rank[s * N_CORES]] + 7) // 8 * 8) or 8
                 for s in range(NBLK))
    base = np.concatenate([[0], np.cumsum(caps)]).astype(int)
    sum_caps = int(base[-1])

    pad_idx = np.full((N_CORES, sum_caps), -1, np.int64)
    for s in range(NBLK):
        for c in range(N_CORES):
            t = rank[s * N_CORES + c]
            ids = order[offs[t]:offs[t + 1]]
            pad_idx[c, base[s]:base[s] + len(ids)] = ids

    blk_of = rank.reshape(NBLK, N_CORES).T      # [core, slot]

    # rows where any clipped count would corrupt the result -> host recompute
    over_rows = np.unique(np.argwhere(cnt > 15)[:, 0])
    return cnt, pad_idx, caps, blk_of, over_rows


def _merged_tables(pieces, ranks, files, tiles, mask):
    p = np.asarray(pieces, np.float32)
    r = np.asarray(ranks, np.float32)
    f = np.asarray(files, np.float32)
    t = np.asarray(tiles, np.float32)
    m = np.asarray(mask, np.float32)
    return (t + (p + r + f) * m).reshape(64, PIECE, DOUT)


def _make_tab(merged, blk_of):
    """Per-core [128, NBLK*TABW] fp16, tile (s, h*6+j)[p, d] =
    255 * merged[blk(c,s), j*128+p, h*128+d]."""
    planes = (merged * OSCALE).reshape(64, 6, 128, 2, 128)
    planes = planes.astype(np.float16)
    tabs = []
    for c in range(N_CORES):
        tc_ = planes[blk_of[c]]                    # [8, 6, 128, 2, 128]
        tabs.append(np.ascontiguousarray(
            tc_.transpose(2, 0, 3, 1, 4).reshape(128, -1)))
    return tabs


def _make_cm(cnt_ext, pad_idx, caps):
    """Per-core count planes [128, sum(5*cap)] uint8.  Per slot:
    [plain g0: j0,j1,j3,j4 as fp8 bytes | plain g1 | packed g0: j2+16*j5 |
    packed g1]."""
    base = np.concatenate([[0], np.cumsum(caps)]).astype(int)
    fp8 = ml_dtypes.float8_e4m3
    cms = []
    for c in range(N_CORES):
        parts = []
        for s in range(NBLK):
            cap = caps[s]
            plain, packed = [], []
            for (goff, gn) in _groups(cap):
                ids = pad_idx[c, base[s] + goff:base[s] + goff + gn]
                sel = cnt_ext[ids]                     # [gn, 768] u8
                # [gn, 6, 128] -> [128(p), 6(j), gn(m)]
                t = sel.reshape(gn, 6, 128).transpose(2, 1, 0)
                plain.append(t[:, [0, 1, 3, 4]].astype(fp8).view(np.uint8)
                             .reshape(128, 4 * gn))
                packed.append(t[:, 2] + 16 * t[:, 5])  # [128, gn]
            parts += plain + packed
        cms.append(np.ascontiguousarray(np.concatenate(parts, axis=1)))
    return cms


def _run(inputs, trace=False):
    cnt, pad_idx, caps, blk_of, over_rows = _prep(
        inputs["values"], inputs["lengths"], inputs["kings"])

    merged = _merged_tables(inputs["pieces"], inputs["ranks"],
                            inputs["files"], inputs["tiles"],
                            inputs["factorization_mask"])

    key = caps
    if key not in _prog_cache:
        _prog_cache[key] = _build_program(caps)
    nc = _prog_cache[key]

    nb2 = cnt.shape[0]
    cnt_unclipped = cnt[over_rows].copy() if len(over_rows) else None
    np.minimum(cnt, 15, out=cnt, casting="unsafe")
    cnt_ext = np.zeros((nb2 + 1, PIECE), np.uint8)
    cnt_ext[:nb2] = cnt

    tabs = _make_tab(merged, blk_of)
    cms = _make_cm(cnt_ext, pad_idx, caps)
    in_maps = [{"tab": tabs[c], "cm": cms[c]} for c in range(N_CORES)]

    res = run_bass_kernel_spmd(nc, in_maps, list(range(N_CORES)),
                               trace=trace)

    base = np.concatenate([[0], np.cumsum(caps)]).astype(int)
    comb = np.zeros((nb2, DOUT), np.float32)
    for c in range(N_CORES):
        flat = res.results[c]["out"]               # [128, 2*sum_caps] u8
        for s in range(NBLK):
            cap = caps[s]
            # [128, 2, cap] -> [cap, 2, 128] -> [cap, 256]
            rows = (flat[:, 2 * base[s]:2 * (base[s] + cap)]
                    .astype(np.float32)
                    .reshape(128, 2, cap).transpose(2, 1, 0)
                    .reshape(cap, DOUT))
            ids = pad_idx[c, base[s]:base[s] + cap]
            valid = ids >= 0
            comb[ids[valid]] = rows[valid]
    comb *= np.float32(1.0 / OSCALE)
    np.clip(comb, 0.0, 1.0, out=comb)

    if len(over_rows):
        # counts were clipped at 15 on device for these rows: recompute f32
        kings = np.asarray(inputs["kings"]).astype(np.int64)
        row_blk = np.concatenate([kings[:, 0], _PERM[kings[:, 1]]])
        for i, r in enumerate(over_rows):
            comb[r] = np.clip(
                cnt_unclipped[i].astype(np.float32) @ merged[row_blk[r]],
                0.0, 1.0)

    return (comb[:B], comb[B:]), res


def kernel(**inputs):
    (a, b), _ = _run(inputs, trace=False)
    return a, b


# revision 33
# speedup vs baseline: 1.0169x; 1.0068x over previous
"""Trainium2 kernel for the NNUE-style factorized embedding segment-sum.

Strategy ("flip"): the ragged two-table embedding-bag is reformulated as
block matmuls with the TABLE STATIONARY and the per-bag count columns as the
moving operand.  For output row (bag) with king-square block t, the
gather+segment-sum over its ragged feature ids equals
merged_table[t] slices.T @ count_cols, computed as 6 contraction chunks
(768 = 6*128 features) x 2 dout halves accumulating in
PSUM[dout_half=128, items].

Wins over the chunked lhsT=counts formulation (42.7us baseline):
 - items stream as matmul COLUMNS, so per-(core,slot) capacity is the max
   block size rounded to 8, not to 128: ~2% padding instead of ~12.5%, and
   the PE does 12 cycles/item (the dense floor: 768*256/128^2) -> ~21us.
 - counts ride as fp8e4 (ints <= 15 exact; the Poisson(0.04) counts max at
   3, anything above is clipped and those rows recomputed on host), with
   one stream pair (j2, j5) nibble-packed to shave HBM bytes: whole-slot
   uint32 bitwise split on DVE (only DVE passes the bitVec engine check)
   + one u8->fp8 cast on ACT.  Packing more pairs does not fit: Pool
   casts at 2.5ns/elem, ACT at 1.2, and DVE is full with PSUM drains.
 - outputs ride to HBM as uint8 (tables pre-scaled by 255, PSUM clipped to
   [0,255] on the DVE drain, host divides back): total chip HBM traffic
   ~46MB, well under the chip ceiling for the ~22us window.
 - dummy matmuls from t~0 keep the PE busy through the DMA fill so the HAM
   clock gate (1.2 -> 2.4 GHz after one fully-busy ~3.4us window) flips
   early in the stream instead of ~8us in.

Host (integer work only): merge factor tables (tiles+(pieces+ranks+files)
*mask -> [64,768,256] fp16), build per-bag count rows in merged-table space
(output b columns flip-remapped so ONE table serves both outputs), sort the
64 blocks by descending item count and deal rank r -> (slot r//8, core r%8)
so each slot's shared SPMD capacity is tight, pack counts, unscramble
outputs.

Device per slot: load table tile (ACT ring) + packed counts (SP ring,
issued two slots ahead, paired for line rate), 2 unpack ops per item group,
24 matmuls (2 item groups x 2 dout halves x 6 feature chunks), clipped
PSUM->u8 drains on DVE, stores batched per slot-pair mid-kernel with the
final slot's pieces split tiny across both rings (the exit barrier gates on
their HBM write receipts).
"""

import numpy as np
import ml_dtypes

import concourse.bass as bass
import concourse.tile as tile
from concourse import bacc, mybir
from concourse.bass_utils import run_bass_kernel_spmd

N_CORES = 8
B = 16384          # bags
KPL = 12           # piece planes
DOUT = 256
PIECE = 768        # KPL * 64
NBLK = 8           # table blocks (slots) per core: 64 king squares / 8 cores
TABW = 12 * 128    # table tile cols per slot: (2 dout halves x 6 fchunks) x 128
OSCALE = 255.0     # uint8 output quantization scale (folded into the table)

# ---------------------------------------------------------------------------
# host-side integer prep tables
_sq = np.arange(64)
_PERM = (7 - _sq // 8) * 8 + _sq % 8          # vertical king-square flip
_v = np.arange(PIECE)
_vk, _vr, _vf = _v // 64, (_v % 64) // 8, _v % 8
_FLIP_COL = ((_vk + 6) % 12) * 64 + (7 - _vr) * 8 + _vf

_prog_cache = {}

NDUMMY = 38        # PE warmup matmuls riding the DMA fill window


def _groups(cap):
    g0 = (cap // 2 + 7) // 8 * 8
    return [(0, g0), (g0, cap - g0)]


def _build_program(caps: tuple):
    """Bass program for one core (SPMD across all 8).

    caps[s] = item capacity of slot s (multiple of 8, shared by all cores).
    """
    nc = bacc.Bacc("TRN2", target_bir_lowering=False, debug=False)
    f32 = mybir.dt.float32
    fp16 = mybir.dt.float16
    fp8 = mybir.dt.float8e4
    u8 = mybir.dt.uint8
    u32 = mybir.dt.uint32
    A = mybir.AluOpType

    sum_caps = sum(caps)
    cm_w = [5 * c for c in caps]       # count bytes per slot (4 plain + 1 packed)
    base = np.concatenate([[0], np.cumsum(caps)]).astype(int)
    cbase = np.concatenate([[0], np.cumsum(cm_w)]).astype(int)
    maxcap = max(caps)

    tab = nc.dram_tensor("tab", [128, NBLK * TABW], fp16,
                         kind="ExternalInput").ap()
    # cm[p, cbase(s) + goff*3 + jj*gn + m] = count[item m, feat jj*128+p]
    #   + 16*count[item m, feat (jj+3)*128+p]   (group-major within a slot)
    cm = nc.dram_tensor("cm", [128, int(cbase[-1])], u8,
                        kind="ExternalInput").ap()
    # out[p, 2*base(s) + h*cap + m] = round(255*clip(result))[item m, h*128+p]
    out = nc.dram_tensor("out", [128, 2 * sum_caps], u8,
                         kind="ExternalOutput").ap()

    with tile.TileContext(nc) as tc:
        with (
            tc.tile_pool(name="tabp", bufs=5) as tabp,
            tc.tile_pool(name="cmp", bufs=4) as cmp_,
            tc.tile_pool(name="cmup", bufs=3) as cmup,
            tc.tile_pool(name="cmtp", bufs=3) as cmtp,
            tc.tile_pool(name="outp", bufs=4) as outp,
            tc.tile_pool(name="warmp", bufs=1) as wmp,
            tc.tile_pool(name="ps", bufs=8, space="PSUM") as psp,
        ):
            # ---- PE warmup: HAM clock gate needs ~3.4us of sustained busy
            # time to lift 1.2 -> 2.4 GHz; burn the DMA fill window on dummy
            # matmuls so the real stream runs warm.  Memsets ride DVE (idle
            # at start - GpSimd's queue carries framework memsets).
            wl = wmp.tile([128, 128], fp16, tag="warml")
            wr = wmp.tile([128, 128], fp16, tag="warmr")
            nc.vector.memset(wl[:], 0)
            nc.vector.memset(wr[:], 0)
            wp = psp.tile([128, 512], f32, tag="ps")
            for _ in range(NDUMMY):
                nc.tensor.matmul(wp[:, :128], lhsT=wl[:], rhs=wr[:])

            # ---- DMA plan: tables on the ACT HWDGE ring, packed counts on
            # the SP ring.  Slot 0 splits at the dout-half / item-group
            # boundary so the first matmuls wait on less data; later slots
            # load in pairs (bigger DMAs run closer to line rate, half the
            # ring issue slots).
            tts = {}

            def load_tab(s0, nslots=1, split=1):
                tt = tabp.tile([128, nslots * TABW], fp16, tag="tab")
                w = nslots * TABW
                bnds = [w * k // split // 128 * 128 for k in range(split + 1)]
                for k in range(split):
                    nc.scalar.dma_start(
                        tt[:, bnds[k]:bnds[k + 1]],
                        tab[:, s0 * TABW + bnds[k]:s0 * TABW + bnds[k + 1]])
                for q in range(nslots):
                    tts[s0 + q] = (tt, q * TABW)

            cms = {}

            def load_cm(s0, nslots=1, bnds=None, order=None, ring=None):
                w = int(cbase[s0 + nslots] - cbase[s0])
                ct = cmp_.tile([128, w], u8, tag="cm")
                if bnds is None:
                    bnds = [0, w]
                pieces = list(range(len(bnds) - 1))
                if order is not None:
                    pieces = order
                if ring is None:
                    ring = nc.sync
                for k in pieces:
                    ring.dma_start(
                        ct[:, bnds[k]:bnds[k + 1]],
                        cm[:, int(cbase[s0]) + bnds[k]:
                           int(cbase[s0]) + bnds[k + 1]])
                off = 0
                for q in range(nslots):
                    cms[s0 + q] = (ct, off)
                    off += cm_w[s0 + q]

            g0n = _groups(caps[0])[0][1]
            c0 = caps[0]
            load_tab(0, split=2)
            load_cm(0, bnds=[0, 4 * g0n, 4 * c0, 5 * c0],
                    order=[0, 2, 1])
            load_tab(1)
            # cm1 rides the ACT ring: during the fill the SP ring already
            # queues all of cm0, so balancing slot 1's bytes across rings
            # lands them ~1.5us earlier
            load_cm(1, ring=nc.scalar)

            outt = None
            for s in range(NBLK):
                cap = caps[s]
                tt, toff = tts[s]
                ct, coff = cms[s]

                # ---- unpack the (j2, j5) nibble pair.  The other four
                # streams ride as plain fp8 straight from HBM (engines lack
                # the elementwise budget to cast more than ~one stream per
                # slot: Pool casts at 2.5ns/elem, ACT at 1.2, DVE is full
                # with drains).  Whole-slot uint32 bitwise split on DVE
                # (only DVE passes the bitVec engine check), one u8->fp8
                # cast on ACT over both nibble streams.
                cmu = cmup.tile([128, 2 * maxcap], u8, tag="cmu")
                cmt = cmtp.tile([128, 2 * maxcap], fp8, tag="cmt")
                x32 = ct[:, coff + 4 * cap:coff + 5 * cap].bitcast(u32)
                nc.vector.tensor_scalar(cmu[:, 0:cap].bitcast(u32), x32,
                                        0x0F0F0F0F, None, A.bitwise_and)
                nc.vector.tensor_scalar(cmu[:, cap:2 * cap].bitcast(u32),
                                        x32, 4, 0x0F0F0F0F,
                                        A.logical_shift_right,
                                        A.bitwise_and)
                # j2 cast on DVE (needed by the 5th matmul of each
                # group; ACT's queue carries the DMA issues and would
                # deliver it late), j5 cast on ACT (needed by the 6th)
                nc.vector.tensor_scalar(cmt[:, :cap], cmu[:, :cap],
                                        1.0, None, A.mult)
                nc.scalar.activation(cmt[:, cap:2 * cap],
                                     cmu[:, cap:2 * cap],
                                     mybir.ActivationFunctionType.Copy)

                # prefetch (paired), two slots ahead
                ns = max(tts.keys()) + 1
                while ns < NBLK and ns <= s + 3:
                    k = min(2, NBLK - ns)
                    load_tab(ns, nslots=k)
                    load_cm(ns, nslots=k)
                    ns += k

                # ---- matmuls: table tile stationary, count cols moving;
                # slot-pair-shared out tile (even slot first half)
                if s % 2 == 0:
                    pair_w = 2 * (cap + (caps[s + 1] if s + 1 < NBLK else 0))
                    outt = outp.tile([128, 2 * 2 * maxcap], u8, tag="out")
                    ooff = 0
                else:
                    ooff = 2 * caps[s - 1]
                g0n_ = _groups(cap)[0][1]
                pbase = [0, 4 * g0n_]      # plain-region group offsets
                qbase = [0, g0n_]          # nibble-region group offsets
                PLAIN_IDX = {0: 0, 1: 1, 3: 2, 4: 3}
                # slot 0 runs group-outer so the last-loaded plain-g1
                # piece is not needed until halfway through the slot
                hg = ([(h, gg) for h in (0, 1)
                       for gg in enumerate(_groups(cap))] if s > 0 else
                      [(h, gg) for gg in enumerate(_groups(cap))
                       for h in (0, 1)])
                for h, (gi, (goff, gn)) in hg:
                        ps = psp.tile([128, 512], f32, tag="ps")
                        # plain streams first: the packed pair's unpack
                        # chain (load -> DVE bitwise -> cast) gets four
                        # matmuls of cover
                        for idx, j in enumerate((0, 1, 3, 4, 2, 5)):
                            if j in PLAIN_IDX:
                                cb = coff + pbase[gi] + PLAIN_IDX[j] * gn
                                rhs = ct[:, cb:cb + gn].bitcast(fp8)
                            else:
                                cb = (qbase[gi] if j == 2
                                      else cap + qbase[gi])
                                rhs = cmt[:, cb:cb + gn]
                            nc.tensor.matmul(
                                ps[:, :gn],
                                lhsT=tt[:, toff + (h * 6 + j) * 128:
                                        toff + (h * 6 + j + 1) * 128],
                                rhs=rhs,
                                start=(idx == 0),
                                stop=(idx == 5),
                            )
                        # clip(psum, 0, 255) -> u8 out tile (table carries
                        # the 255x scale; host divides back)
                        dsl = outt[:, ooff + h * cap + goff:
                                   ooff + h * cap + goff + gn]
                        nc.vector.tensor_scalar(dsl, ps[:, :gn],
                                                255.0, 0.0, A.min, A.max)
                        if s == 7 and h == 0 and gi == 1:
                            # final slot: h0 half fully drained - store it
                            # now so only the small h1 pieces gate the exit
                            ob7 = 2 * int(base[s])
                            nc.scalar.dma_start(
                                out[:, ob7:ob7 + cap],
                                outt[:, ooff:ooff + cap])

                # ---- stores: batched per slot pair for slots 0-5 (rings
                # alternate), singles for 6 and 7 with the final pieces
                # split small across both rings so their HBM write receipts
                # (which gate the exit barrier) overlap and cover little
                # data
                if s in (1, 3, 5):
                    w = 2 * (caps[s - 1] + cap)
                    ob = 2 * int(base[s - 1])
                    ring = nc.scalar if s in (1, 5) else nc.sync
                    ring.dma_start(out[:, ob:ob + w], outt[:, :w])
                elif s == 6:
                    ob = 2 * int(base[s])
                    nc.sync.dma_start(out[:, ob:ob + 2 * cap],
                                      outt[:, :2 * cap])
                elif s == 7:
                    ob = 2 * int(base[s])
                    cut = 2 * cap - 64
                    nc.sync.dma_start(out[:, ob + cap:ob + cut],
                                      outt[:, ooff + cap:ooff + cut])
                    nc.scalar.dma_start(out[:, ob + cut:ob + 2 * cap],
                                        outt[:, ooff + cut:ooff + 2 * cap])

    nc.compile()
    return nc


def _prep(values, lengths, kings):
    """Host prep: counts in merged-table column space, block assignment."""
    values = np.asarray(values).astype(np.int64)
    lengths = np.asarray(lengths).astype(np.int64)
    kings = np.asarray(kings).astype(np.int64)
    nb = lengths.shape[0]

    seg = np.repeat(np.arange(nb, dtype=np.int64), lengths)
    cnt_a = np.bincount(seg * PIECE + values,
                        minlength=nb * PIECE).reshape(nb, PIECE)
    cnt_b = np.bincount(seg * PIECE + _FLIP_COL[values],
                        minlength=nb * PIECE).reshape(nb, PIECE)
    cnt = np.concatenate([cnt_a, cnt_b])        # [2B, PIECE] ints

    blk = np.concatenate([kings[:, 0], _PERM[kings[:, 1]]])
    sizes = np.bincount(blk, minlength=64)
    order = np.argsort(blk, kind="stable")
    offs = np.concatenate([[0], np.cumsum(sizes)])

    rank = np.argsort(-sizes, kind="stable")    # block ids, desc size
    caps = tuple(int((sizes[rank[s * N_CORES]] + 7) // 8 * 8) or 8
                 for s in range(NBLK))
    base = np.concatenate([[0], np.cumsum(caps)]).astype(int)
    sum_caps = int(base[-1])

    pad_idx = np.full((N_CORES, sum_caps), -1, np.int64)
    for s in range(NBLK):
        for c in range(N_CORES):
            t = rank[s * N_CORES + c]
            ids = order[offs[t]:offs[t + 1]]
            pad_idx[c, base[s]:base[s] + len(ids)] = ids

    blk_of = rank.reshape(NBLK, N_CORES).T      # [core, slot]

    # rows where any clipped count would corrupt the result -> host recompute
    over_rows = np.unique(np.argwhere(cnt > 15)[:, 0])
    return cnt, pad_idx, caps, blk_of, over_rows


def _merged_tables(pieces, ranks, files, tiles, mask):
    p = np.asarray(pieces, np.float32)
    r = np.asarray(ranks, np.float32)
    f = np.asarray(files, np.float32)
    t = np.asarray(tiles, np.float32)
    m = np.asarray(mask, np.float32)
    return (t + (p + r + f) * m).reshape(64, PIECE, DOUT)


def _make_tab(merged, blk_of):
    """Per-core [128, NBLK*TABW] fp16, tile (s, h*6+j)[p, d] =
    255 * merged[blk(c,s), j*128+p, h*128+d]."""
    planes = (merged * OSCALE).reshape(64, 6, 128, 2, 128)
    planes = planes.astype(np.float16)
    tabs = []
    for c in range(N_CORES):
        tc_ = planes[blk_of[c]]                    # [8, 6, 128, 2, 128]
        tabs.append(np.ascontiguousarray(
            tc_.transpose(2, 0, 3, 1, 4).reshape(128, -1)))
    return tabs


def _make_cm(cnt_ext, pad_idx, caps):
    """Per-core count planes [128, sum(5*cap)] uint8.  Per slot:
    [plain g0: j0,j1,j3,j4 as fp8 bytes | plain g1 | packed g0: j2+16*j5 |
    packed g1]."""
    base = np.concatenate([[0], np.cumsum(caps)]).astype(int)
    fp8 = ml_dtypes.float8_e4m3
    cms = []
    for c in range(N_CORES):
        parts = []
        for s in range(NBLK):
            cap = caps[s]
            plain, packed = [], []
            for (goff, gn) in _groups(cap):
                ids = pad_idx[c, base[s] + goff:base[s] + goff + gn]
                sel = cnt_ext[ids]                     # [gn, 768] u8
                # [gn, 6, 128] -> [128(p), 6(j), gn(m)]
                t = sel.reshape(gn, 6, 128).transpose(2, 1, 0)
                plain.append(t[:, [0, 1, 3, 4]].astype(fp8).view(np.uint8)
                             .reshape(128, 4 * gn))
                packed.append(t[:, 2] + 16 * t[:, 5])  # [128, gn]
            parts += plain + packed
        cms.append(np.ascontiguousarray(np.concatenate(parts, axis=1)))
    return cms


def _run(inputs, trace=False):
    cnt, pad_idx, caps, blk_of, over_rows = _prep(
        inputs["values"], inputs["lengths"], inputs["kings"])

    merged = _merged_tables(inputs["pieces"], inputs["ranks"],
                            inputs["files"], inputs["tiles"],
                            inputs["factorization_mask"])

    key = caps
    if key not in _prog_cache:
        _prog_cache[key] = _build_program(caps)
    nc = _prog_cache[key]

    nb2 = cnt.shape[0]
    cnt_unclipped = cnt[over_rows].copy() if len(over_rows) else None
    np.minimum(cnt, 15, out=cnt, casting="unsafe")
    cnt_ext = np.zeros((nb2 + 1, PIECE), np.uint8)
    cnt_ext[:nb2] = cnt

    tabs = _make_tab(merged, blk_of)
    cms = _make_cm(cnt_ext, pad_idx, caps)
    in_maps = [{"tab": tabs[c], "cm": cms[c]} for c in range(N_CORES)]

    res = run_bass_kernel_spmd(nc, in_maps, list(range(N_CORES)),
                               trace=trace)

    base = np.concatenate([[0], np.cumsum(caps)]).astype(int)
    comb = np.zeros((nb2, DOUT), np.float32)
    for c in range(N_CORES):
        flat = res.results[c]["out"]               # [128, 2*sum_caps] u8
        for s in range(NBLK):
            cap = caps[s]
            # [128, 2, cap] -> [cap, 2, 128] -> [cap, 256]
            rows = (flat[:, 2 * base[s]:2 * (base[s] + cap)]
                    .astype(np.float32)
                    .reshape(128, 2, cap).transpose(2, 1, 0)
                    .reshape(cap, DOUT))
            ids = pad_idx[c, base[s]:base[s] + cap]
            valid = ids >= 0
            comb[ids[valid]] = rows[valid]
    comb *= np.float32(1.0 / OSCALE)
    np.clip(comb, 0.0, 1.0, out=comb)

    if len(over_rows):
        # counts were clipped at 15 on device for these rows: recompute f32
        kings = np.asarray(inputs["kings"]).astype(np.int64)
        row_blk = np.concatenate([kings[:, 0], _PERM[kings[:, 1]]])
        for i, r in enumerate(over_rows):
            comb[r] = np.clip(
                cnt_unclipped[i].astype(np.float32) @ merged[row_blk[r]],
                0.0, 1.0)

    return (comb[:B], comb[B:]), res


def kernel(**inputs):
    (a, b), _ = _run(inputs, trace=False)
    return a, b
